# revision 95
# baseline (speedup 1.0000x reference)
"""Causal attention kernel for Trainium2, 8 NeuronCores.

Problem: x[4,4096,768] f32; Wq/Wk/Wv [768,64] f32.
  q,k,v = x@W*; S = q@k.T (causal); out = softmax(S/8)@v  -> [4,4096,64] f32.

Sharding: data-parallel over batch (4) x interleaved q-chunk split (2).
  The 8 query chunks of 512 rows are split A={0,3,4,7}, B={1,2,5,6};
  both halves get exactly half the causal score area and identical
  exp-instruction counts, so the two programs balance.
  Cores 0-3 run program A (batches 0-3), cores 4-7 run program B.

Device algorithm (per core), v3 (token-major PV, 2-slot proj ring):
  - load xT (host-transposed) [768, NK] bf16 in column waves; slot
    order puts the first kv quarter-wave, the wm constants and the
    first chunk's q wave ahead of everything else so the exp pipeline
    starts as early as the DMA stream allows.
  - projections on PE in bf16 (kv: M=128 [Wv|Wk], q: M=64), 6
    contraction passes per group, into a TWO-slot PSUM ring so the
    next group's projection overlaps this group's consumers.  Each kv
    group: ONE combined [128,g] PSUM->SBUF staging copy on DVE
    (engine cost is free-size only, so combining is free; it also
    releases the ring slot), then both fp8e4 DoubleRow k-casts on
    gpsimd off the DVE queue.  Both q casts are DVE-direct from PSUM:
    each chunk's q gates a stretch of the exp stream and the DVE
    queue has slack when its wave lands.
  - v is transposed to token-major ON PE (identity matmul into a bf16
    PSUM tile in the proj ring, Ldweights is free) instead of a DMA
    transpose: the DMA engine mutex is owned by the serial x-wave
    stream for the first ~18us, so a DMA-path transpose would stall
    PV and starve ACT.  The transpose is deferred to the next group's
    emission so a q projection can slip into the ring first.
  - scores transposed: ST[j,i] per (key tile 128 x q chunk 512) via ONE
    fp8 DoubleRow matmul per key tile (0.5 cyc/row).  fp8 rounding of
    q/k costs ~1.5% rms on the output - the only sub-bf16 step.
  - P = exp(S/8) on ACT to bf16 (no max subtraction: |S/8| <= ~7);
    the causal mask is FUSED into the score matmul: one extra
    identity-stationary matmul per diagonal tile accumulates
    -240*tril(ones,-1) onto the diagonal score block, so exp() itself
    zeroes the above-diagonal region (nothing on the exp->PV path).
  - PV is TOKEN-MAJOR: per (key tile t, token block b) a matmul with
    the P subtile [128 keys, 128 tok] STATIONARY and vx [128 keys, 65]
    moving -> o[128 tok, 65] accumulated in PSUM over t.  Matmul cost
    is output-free-size only, so this halves PV PE time, and the
    softmax denominator lands in PSUM col 64 per block (ones column
    of vx).  Only the chunk's first matmul sets start: psum zeroing
    is per 2KB zero region, later blocks land on pending-zero.
  - normalize: r = 1/denom via reciprocal_approx_fast on the strided
    PSUM denominator columns, then one broadcast tensor_tensor mult
    (DVE) straight from PSUM, then token-major DMA out [512, 64] f32;
    finishes run immediately at chunk completion, and the last
    chunk's finish is split in half so the tail is short.
  - schedule: one flat step list.  Phase 1 runs the small chunk
    entirely first (its first step SPLIT into two 256-token q halves
    so exp starts before the full q wave lands), then the big chunk
    with its pairs ordered by their kv wave's ARRIVAL (accumulation
    order is free after the start matmul), so pairs with resident
    keys run while later waves stream in.  Phase 2 merges its two
    chunks in ascending tile order with the big chunk lagged 2 pairs.
    kv groups are emitted in wave-arrival order with a 2-step
    required lookahead plus a 6-step arrived-early window; q
    projections are emitted at precomputed step indices matching
    their wave arrival.  PSUM: proj ring 2 + S 2x2 + o 2 = 8 banks.
  - a dependency-gated chain of tiny warmup matmuls keeps the cost
    model's PE p-state clock running so the real DMA-gated matmuls
    price at ramped speed.
"""

import math

import numpy as np
import ml_dtypes

import concourse.bass as bass
import concourse.bacc as bacc
import concourse.mybir as mybir
import concourse.tile as tile
from concourse.bass_utils import run_bass_kernel_spmd
from concourse.tile_rust import add_dep_helper

B, N, D_IN, D_OUT = 4, 4096, 768, 64
CHUNKS_A = [0, 3, 4, 7]
CHUNKS_B = [1, 2, 5, 6]
NDC = D_IN // 128  # 6 contraction chunks
F8 = mybir.dt.float8e4
BF16 = mybir.dt.bfloat16
F32 = mybir.dt.float32
DR = mybir.MatmulPerfMode.DoubleRow
SCALE = 0.125  # 1/sqrt(64)
WM_W = NDC * 192 + 128 + 128  # [wqkv | Bmask | I128]
STEP_NS = 1040.0  # ACT exp per full step


def _wave_order(chunks, NK):
    """512-col x-wave order: per chunk (processing order): its q-wave
    first (if not yet loaded), then any kv waves it needs, ascending."""
    waves = []
    loaded = set()

    def add(w):
        if w not in loaded and 512 * w < NK:
            waves.append(w)
            loaded.add(w)

    for c in chunks:
        add(c)  # q-wave = columns [512c, 512c+512)
        for w in range(c + 1):
            add(w)
    return waves


def build_half(chunks, debug_dump=False):
    NQ = 512 * len(chunks)
    T_need = 4 * (max(chunks) + 1)
    NK = 128 * T_need
    nkt = T_need
    nc = bacc.Bacc("TRN2", target_bir_lowering=False, debug=False)

    xT_d = nc.dram_tensor("xT", [D_IN, NK], BF16, kind="ExternalInput")
    # [wqkv 6*192 | Bmask 128 | I128 128] bf16; wqkv dc-slices [Wq|Wv|Wk]
    wm_d = nc.dram_tensor("wm", [128, WM_W], BF16, kind="ExternalInput")
    o_d = nc.dram_tensor("o", [NQ, D_OUT], F32, kind="ExternalOutput")

    from contextlib import ExitStack

    with tile.TileContext(nc) as tc, ExitStack() as stk:
        cpool = stk.enter_context(tc.tile_pool(name="const", bufs=1))
        xpool = stk.enter_context(tc.tile_pool(name="xt", bufs=1))
        jpool = stk.enter_context(tc.tile_pool(name="proj", bufs=1))
        ppool = stk.enter_context(tc.tile_pool(name="pp", bufs=3))
        fpool = stk.enter_context(tc.tile_pool(name="fin", bufs=3))

        # ---- constants / inputs ----
        w_sb = cpool.tile([128, WM_W], BF16, tag="wm")
        w3 = w_sb[:, 0 : NDC * 192].rearrange("p (c j) -> p c j", j=192)
        # Bmask[p, j] = -240 for p > j: accumulated onto diagonal score
        # blocks via one identity-stationary matmul, so exp() itself
        # zeroes the above-diagonal region (no DVE mask on the
        # exp->PV critical path)
        bm_sb = w_sb[:, NDC * 192 : NDC * 192 + 128]
        i128_sb = w_sb[:, NDC * 192 + 128 :]
        i64_sb = w_sb[0:64, NDC * 192 + 128 : NDC * 192 + 128 + 64]

        zbias = cpool.tile([128, 1], F32, tag="zbias")
        nc.vector.memset(zbias[:, :], 0.0)
        warm_sb = cpool.tile([1, 64], BF16, tag="warm")
        nc.vector.memset(warm_sb[:, :], 0.0)

        xt_sb = xpool.tile([128, NDC * NK], BF16, tag="xt")
        xt3 = xt_sb.rearrange("p (c n) -> p c n", n=NK)
        xT3d = xT_d.ap().rearrange("(c p) n -> p c n", p=128)

        # ---- DMA slot order: [w0a, wm, (phase-1 q waves if not wave 0),
        # w0b, remaining wave order].  Quarter-split of wave 0 lets the
        # first kv group start right after wm. ----
        order = _wave_order(chunks, NK)
        slots = [("x", (0, 256)), ("wm", None)]
        placed = {0}
        if chunks[0] != 0:
            # first chunk's q wave, quarter-split so its q projection can
            # start after the first half lands
            qw = chunks[0]
            slots += [("x", (512 * qw, 256)), ("x", (512 * qw + 256, 256))]
            placed.add(qw)
        slots.append(("x", (256, 256)))
        for w in order:
            if w not in placed:
                slots.append(("x", (512 * w, 512)))
                placed.add(w)
        col_eta = {}
        t_acc = 2300.0
        for kind, payload in slots:
            if kind == "wm":
                t_acc += 940.0
                continue
            g0, g = payload
            t_acc += 2185.0 * g / 512.0
            for c0 in range(g0, g0 + g, 256):
                col_eta[c0] = t_acc + 950.0  # + DMA sem propagation
        for kind, payload in slots:
            if kind == "wm":
                nc.sync.dma_start(w_sb[:, :], wm_d.ap())
            else:
                g0, g = payload
                nc.sync.dma_start(
                    xt3[:, :, g0 : g0 + g], xT3d[:, :, g0 : g0 + g]
                )

        def wave_eta_ms(col):
            # estimated arrival of the wave containing `col` (scheduler
            # hint only: stops the scheduler's DMA-blind model from
            # hoisting projections ahead of attention work)
            return col_eta[(col // 256) * 256] / 1e6

        # per-chunk q emission step index (global step counter) from the
        # arrival model: emit the q projection slightly before its wave
        # lands so the PE picks it up without head-of-line blocking
        q_arrival = {c: col_eta[512 * c + 256] for c in chunks}
        est_first_exp = q_arrival[chunks[0]] + 1700.0
        q_emit_step = {
            c: max(
                0,
                math.ceil((q_arrival[c] - 700.0 - est_first_exp) / STEP_NS),
            )
            for c in chunks
        }
        q_order = sorted(chunks, key=lambda c: q_arrival[c])

        # kv groups in WAVE-ARRIVAL order with arrival-model step
        # indices: a group whose wave has landed is emitted even if it
        # is ahead of the column-order need, so its proj/copy/cast chain
        # never queues behind a group still waiting on the DMA stream
        kv_groups = []
        g0 = 0
        while g0 < NK:
            g = min(256 if g0 < 512 else 512, NK - g0)
            arr = col_eta[g0 + g - 256]
            kv_groups.append(
                [
                    arr,
                    max(0, math.ceil((arr - est_first_exp) / STEP_NS)),
                    g0,
                    g,
                    False,
                ]
            )
            g0 += g
        kv_groups.sort(key=lambda e: (e[0], e[2]))

        # ---- projection targets ----
        kv_sb = jpool.tile([128, NK], BF16, tag="kv")  # [vT; kT] e-major
        vx_sb = jpool.tile([128, nkt * 65], BF16, tag="vx")
        vx3 = vx_sb.rearrange("p (t e) -> p t e", e=65)
        nc.gpsimd.memset(vx3[:, :, 64:65], 1.0)  # denominator ones column
        kt_sb = jpool.tile([32, nkt * 256], F8, tag="kt")
        kt4 = kt_sb.rearrange("p (t j m) -> p t j m", j=2, m=128)
        qt_sb = jpool.tile([32, 2 * NQ], F8, tag="qt")
        qt3 = qt_sb.rearrange("p (j n) -> p j n", j=2)

        pref = {}
        done = {"kv": 0, "q": set()}

        def flush_vt():
            # deferred v->token-major transpose of the last kv group (PE
            # identity transpose into the ring, bf16 PSUM out): deferring
            # keeps the ring slot free for a q projection right after the
            # group's staging copy
            pend = done.pop("vt", None)
            if pend is None:
                return
            g0, g = pend
            t0, t1 = g0 // 128, (g0 + g) // 128
            nt = t1 - t0
            vt = pref["proj"].tile([128, nt * 64], BF16, tag="proj", name="vt")
            for j in range(nt):
                nc.tensor.matmul(
                    vt[:, 64 * j : 64 * j + 64],
                    lhsT=kv_sb[0:64, g0 + 128 * j : g0 + 128 * (j + 1)],
                    rhs=i64_sb,
                    is_transpose=True,
                    start=True,
                    stop=True,
                    skip_group_check=True,
                )
            nc.vector.tensor_copy(
                vx3[:, t0:t1, 0:64],
                vt.rearrange("p (t e) -> p t e", e=64),
            )

        def emit_kv_group(g0, g, dep=None):
          flush_vt()
          with tc.tile_wait_until(wave_eta_ms(g0)):
            t0, t1 = g0 // 128, (g0 + g) // 128
            ps = pref["proj"].tile([128, 512], F32, tag="proj", name="pkv")
            for dc in range(NDC):
                mm = nc.tensor.matmul(
                    ps[:, 0:g],
                    lhsT=w3[:, dc, 64:192],
                    rhs=xt3[:, dc, g0 : g0 + g],
                    start=(dc == 0),
                    stop=(dc == NDC - 1),
                )
                if dep is not None and dc == 0:
                    # throttle: stop the scheduler (whose DMA-blind model
                    # thinks projections are ready early) from hoisting
                    # this group ahead of older attention work
                    add_dep_helper(mm.ins, dep.ins, reason="proj throttle")
            # ONE staging copy to SBUF on DVE (cost is free-size only, so
            # a combined [128,g] copy prices the same as half of it and
            # releases the proj ring slot); fp8 k casts on gpsimd after
            nc.vector.tensor_copy(kv_sb[:, g0 : g0 + g], ps[:, 0:g])
            nc.gpsimd.tensor_copy(kt4[:, t0:t1, 0, :], kv_sb[64:96, g0 : g0 + g])
            nc.gpsimd.tensor_copy(kt4[:, t0:t1, 1, :], kv_sb[96:128, g0 : g0 + g])
            done["vt"] = (g0, g)

        def emit_kv_cols(c_lo, c_hi, dep=None, arrived_only=False):
            # emit unemitted groups overlapping [c_lo, c_hi), scanning in
            # wave-arrival order so a group with resident data never
            # queues behind one still waiting on the DMA stream
            for e in kv_groups:
                arr, _, g0, g, emitted = e
                if emitted or g0 + g <= c_lo or g0 >= c_hi:
                    continue
                if arrived_only and arr > e_now[0]:
                    continue
                emit_kv_group(g0, g, dep=dep)
                e[4] = True

        def emit_kv_upto(tok, dep=None):
            emit_kv_cols(0, min(tok, NK), dep=dep)

        def emit_q_half(ci, qc0, h, ps=None):
          """Project + cast one 256-col half of a q chunk (startup path:
          the half can start as soon as its quarter-wave lands).  `ps`
          may supply a spare PSUM region (e.g. the unused second bank of
          an S half-tile) to keep the proj ring free for kv groups."""
          with tc.tile_wait_until(wave_eta_ms(qc0 + 256 * h)):
            ql0 = 512 * ci + 256 * h
            if ps is None:
                ps = pref["proj"].tile([64, 256], F32, tag="proj", name="pqh")
            for dc in range(NDC):
                nc.tensor.matmul(
                    ps[:, :],
                    lhsT=w3[:, dc, 0:64],
                    rhs=xt3[:, dc, qc0 + 256 * h : qc0 + 256 * h + 256],
                    start=(dc == 0),
                    stop=(dc == NDC - 1),
                )
            nc.vector.tensor_copy(qt3[:, 0, ql0 : ql0 + 256], ps[0:32, :])
            nc.vector.tensor_copy(qt3[:, 1, ql0 : ql0 + 256], ps[32:64, :])

        def emit_q(ci, qc0):
          # steady-state q: one full-width projection; staging copy on
          # DVE (releases the ring slot fast), fp8 casts on gpsimd
          if ci in done["q"]:
              return
          done["q"].add(ci)
          with tc.tile_wait_until(wave_eta_ms(qc0 + 256)):
            ql0 = 512 * ci
            ps = pref["proj"].tile([64, 512], F32, tag="proj", name="pq")
            for dc in range(NDC):
                nc.tensor.matmul(
                    ps[:, :],
                    lhsT=w3[:, dc, 0:64],
                    rhs=xt3[:, dc, qc0 : qc0 + 512],
                    start=(dc == 0),
                    stop=(dc == NDC - 1),
                )
            # both fp8 casts DVE-direct from PSUM: each chunk's q gates a
            # stretch of the exp stream, and the DVE queue has slack when
            # its wave lands — cutting the bf16-staging + gpsimd hop
            # shortens the wave->S chain by ~1us
            nc.vector.tensor_copy(qt3[:, 0, ql0 : ql0 + 512], ps[0:32, :])
            nc.vector.tensor_copy(qt3[:, 1, ql0 : ql0 + 512], ps[32:64, :])

        def q_check(gsi):
            for c in q_order:
                ci = chunks.index(c)
                if ci not in done["q"] and q_emit_step[c] <= gsi:
                    emit_q(ci, 512 * c)

        # ---- psum pools: proj ring 2 + s 2x2 + o 2 = 8 banks ----
        pref["proj"] = stk.enter_context(
            tc.tile_pool(name="ppsum", bufs=2, space="PSUM")
        )
        # p-state warmup chain (see module docstring)
        wsrc = warm_sb
        for wi in range(8):
            warm_ps = pref["proj"].tile([1, 64], F32, tag="proj", name=f"w{wi}")
            nc.tensor.matmul(
                warm_ps[:, :], lhsT=wsrc[:, 0:1], rhs=wsrc[:, :],
                start=True, stop=True,
            )
            wsrc = cpool.tile([1, 64], BF16, tag=f"warm{wi}")
            nc.vector.tensor_copy(wsrc[:, :], warm_ps[:, :])
        spsum = stk.enter_context(tc.tile_pool(name="spsum", bufs=2, space="PSUM"))
        opsum = stk.enter_context(tc.tile_pool(name="opsum", bufs=2, space="PSUM"))

        # ---- attention: two phases, chunks merged by tile order ----
        class Ck:
            def __init__(self, ci, c):
                self.ci, self.c = ci, c
                self.qc0 = 512 * c
                self.ql0 = 512 * ci
                self.T_c = 4 * (c + 1)
                self.npair = self.T_c // 2
                self.remaining = self.npair
                self.o_tile = None
                self.o3 = None
                self.s_cur = None

        def emit_s(ck, pi):
            emit_q(ck.ci, ck.qc0)  # idempotent: q must precede its S
            t0 = 2 * pi
            i0g = max(0, 128 * t0 - ck.qc0)
            s_tile = spsum.tile([128, 1024], F32, tag="s")
            for tl in range(2):
                nc.tensor.matmul(
                    s_tile[:, 512 * tl + i0g : 512 * tl + 512],
                    lhsT=kt4[:, t0 + tl, :, :],
                    rhs=qt3[:, :, ck.ql0 + i0g : ck.ql0 + 512],
                    start=True,
                    stop=True,
                    perf_mode=DR,
                )
                dcol = 128 * (t0 + tl) - ck.qc0
                if 0 <= dcol < 512:  # fused causal mask on diag block
                    nc.tensor.matmul(
                        s_tile[:, 512 * tl + dcol : 512 * tl + dcol + 128],
                        lhsT=i128_sb,
                        rhs=bm_sb,
                        start=False,
                        stop=True,
                        skip_group_check=True,
                    )
            return s_tile

        def emit_s_half(ck, s_half, h):
            # first program step only: scores for one 256-token q half
            # (own [128,512] tile per half so exp h0 has no false dep on
            # the h1 score matmuls)
            for tl in range(2):
                nc.tensor.matmul(
                    s_half[:, 256 * tl : 256 * tl + 256],
                    lhsT=kt4[:, tl, :, :],
                    rhs=qt3[:, :, ck.ql0 + 256 * h : ck.ql0 + 256 * h + 256],
                    start=True,
                    stop=True,
                    perf_mode=DR,
                )
                dcol = 128 * tl - ck.qc0
                if 0 <= dcol < 512 and 256 * h <= dcol < 256 * h + 256:
                    nc.tensor.matmul(
                        s_half[
                            :,
                            256 * tl + dcol - 256 * h : 256 * tl + dcol - 256 * h + 128,
                        ],
                        lhsT=i128_sb,
                        rhs=bm_sb,
                        start=False,
                        stop=True,
                        skip_group_check=True,
                    )

        def emit_fin(ck, blo, bhi):
            """Normalize + store token blocks [blo, bhi) of chunk ck."""
            nb = bhi - blo
            o3 = ck.o3
            r_tile = fpool.tile([128, 4], F32, tag="r", name="r")
            nc.vector.reciprocal_approx_fast(
                r_tile[:, 0:nb], o3[:, blo:bhi, 64]
            )
            n_tile = fpool.tile([128, 256], F32, tag="n", name="n")
            n3 = n_tile.rearrange("p (b e) -> p b e", e=64)
            nc.vector.tensor_tensor(
                n3[:, blo:bhi, :],
                o3[:, blo:bhi, 0:64],
                r_tile[:, 0:nb, None].broadcast_to([128, nb, 64]),
                op=mybir.AluOpType.mult,
            )
            out_ap = o_d.ap()[
                ck.ql0 + 128 * blo : ck.ql0 + 128 * bhi, :
            ].rearrange("(b p) e -> p b e", p=128)
            nc.sync.dma_start(
                out_ap,
                n_tile.rearrange("p (b e) -> p b e", e=64)[:, blo:bhi, :],
            )

        def process(ck, pi, s_next_step):
            """Emit exp/mask/PV for (ck, pi); S for s_next_step emitted
            first so the PE runs ahead of ACT."""
            s_cur = ck.s_cur
            if s_next_step is not None:
                nck, npi = s_next_step
                nck.s_cur = emit_s(nck, npi)
            t0, t1 = 2 * pi, 2 * pi + 1
            i0g = max(0, 128 * t0 - ck.qc0)
            p_tile = ppool.tile([128, 1024], BF16, tag="p")
            p3 = p_tile.rearrange("p (t i) -> p t i", i=512)
            if i0g == 0:
                s_ap, p_ap = s_cur[:, :], p_tile[:, :]
            else:
                s_ap = s_cur.rearrange("p (t i) -> p t i", i=512)[:, :, i0g:512]
                p_ap = p3[:, :, i0g:512]
            exp_inst = nc.scalar.activation(
                p_ap, s_ap, mybir.ActivationFunctionType.Exp,
                bias=zbias[:, :], scale=SCALE,
            )
            for tl, t in ((0, t0), (1, t1)):
                dcol = 128 * t - ck.qc0
                # token-major PV: P subtile stationary, vx moving.
                # start only on the tile's FIRST matmul: start marks the
                # whole 2KB psum zero region, so later blocks' first
                # writes land on pending-zero (one start per bank).
                b_min = max(0, dcol // 128)
                for b in range(b_min, 4):
                    nc.tensor.matmul(
                        ck.o_tile[:, 65 * b : 65 * b + 65],
                        lhsT=p3[:, tl, 128 * b : 128 * b + 128],
                        rhs=vx3[:, t, :],
                        start=(t == 0 and b == 0),
                        stop=(t == 4 * ck.c + b),
                        skip_group_check=True,
                    )
            return exp_inst

        def process_first_split(ck, s_next_step):
            """First program step, pipelined in two 256-token q halves:
            exp of half 0 runs while half 1's q/S chain completes."""
            s_halves = ck.s_cur
            p_tile = ppool.tile([128, 1024], BF16, tag="p")
            p3 = p_tile.rearrange("p (t i) -> p t i", i=512)
            exps = []
            for h in (0, 1):
                if h == 1 and s_next_step is not None:
                    nck, npi = s_next_step
                    nck.s_cur = emit_s(nck, npi)
                exps.append(
                    nc.scalar.activation(
                        p3[:, :, 256 * h : 256 * h + 256],
                        s_halves[h].rearrange("p (t i) -> p t i", i=256),
                        mybir.ActivationFunctionType.Exp,
                        bias=zbias[:, :],
                        scale=SCALE,
                    )
                )
                for tl, t in ((0, 0), (1, 1)):
                    dcol = 128 * t - ck.qc0
                    b_min = max(0, dcol // 128)
                    for b in (2 * h, 2 * h + 1):
                        if b < b_min:
                            continue
                        nc.tensor.matmul(
                            ck.o_tile[:, 65 * b : 65 * b + 65],
                            lhsT=p3[:, tl, 128 * b : 128 * b + 128],
                            rhs=vx3[:, t, :],
                            start=(t == 0 and b == 0),
                            stop=(t == 4 * ck.c + b),
                            skip_group_check=True,
                        )
            return exps

        cks = [Ck(ci, c) for ci, c in enumerate(chunks)]
        # phase 1 sequential: small chunk entirely first — its steps run
        # while the big chunk's q/kv waves are still arriving.  phase 2
        # merged ascending tile order, big chunk lagged 2 pairs (its q
        # wave arrives later); small first on ties.  One flat list so
        # the S pipeline chains across the phase boundary.
        big2 = max(cks[2:4], key=lambda k: k.npair)
        # phase-1 big chunk pairs ordered by their kv wave's ARRIVAL
        # (accumulation order is free once the start matmul ran): pairs
        # whose keys are already resident run while later waves stream in
        p1big = sorted(
            range(cks[1].npair),
            key=lambda p: (col_eta.get(256 * p, 0.0), p),
        )
        steps = (
            [(cks[0], pi) for pi in range(cks[0].npair)]
            + [(cks[1], pi) for pi in p1big]
            + sorted(
                [(ck, pi) for ck in cks[2:4] for pi in range(ck.npair)],
                key=lambda s: (s[1] + (2 if s[0] is big2 else 0), s[0].c),
            )
        )
        for ck in cks:
            ck.o_tile = opsum.tile([128, 260], F32, tag="ot", name=f"o{ck.ci}")
            ck.o3 = ck.o_tile.rearrange("p (b e) -> p b e", e=65)

        # prime the first (split) step
        ck0 = steps[0][0]
        emit_kv_upto(128 * (2 * steps[0][1] + 2))
        done["q"].add(ck0.ci)
        emit_q_half(ck0.ci, ck0.qc0, 0)
        s_h0 = spsum.tile([128, 512], F32, tag="s", name="sh0")
        emit_s_half(ck0, s_h0, 0)
        # h1 q chain before the second kv group: with all-DVE q casts
        # the h1 casts now precede kv0b's staging copy in the DVE queue,
        # so S-h1 fires right behind exp-h0 (h1's projection is routed
        # into the spare second bank of the s_h1 tile, off the ring)
        s_h1 = spsum.tile([128, 1024], F32, tag="s", name="sh1")
        emit_q_half(ck0.ci, ck0.qc0, 1, ps=s_h1[0:64, 512:768])
        emit_kv_upto(512)
        emit_s_half(ck0, s_h1[:, 0:512], 1)
        ck0.s_cur = (s_h0, s_h1[:, 0:512])

        exp_hist = []
        e_now = [0.0]
        for si, (ck, pi) in enumerate(steps):
            nxt = steps[si + 1] if si + 1 < len(steps) else None
            if nxt is not None:
                # two-step kv lookahead (a group's proj/copy/cast chain
                # is ~2.5us), plus arrived-early groups needed within
                # the next six steps
                dep = exp_hist[-9] if len(exp_hist) >= 9 else None
                e_now[0] = est_first_exp + STEP_NS * si
                for k in (1, 2):
                    st = steps[min(si + k, len(steps) - 1)]
                    emit_kv_cols(256 * st[1], 256 * st[1] + 256, dep=dep)
                for k in range(3, 7):
                    st = steps[min(si + k, len(steps) - 1)]
                    emit_kv_cols(
                        256 * st[1],
                        256 * st[1] + 256,
                        dep=dep,
                        arrived_only=True,
                    )
                q_check(si)
            flush_vt()
            if si == 0:
                exp_hist.extend(process_first_split(ck, nxt))
            else:
                exp_hist.append(process(ck, pi, nxt))
            ck.remaining -= 1
            if ck is cks[-1] and ck.remaining == 1:
                # early half-finish of the final chunk: token blocks 0-1
                # are complete after its second-to-last processed pair
                # (processing order is ascending there)
                emit_fin(ck, 0, 2)
                ck.fin_half = True
            if ck.remaining == 0:
                if getattr(ck, "fin_half", False):
                    emit_fin(ck, 2, 4)
                else:
                    emit_fin(ck, 0, 4)

        if debug_dump:
            kt_d = nc.dram_tensor("kt_dump", [32, nkt * 256], F32, kind="ExternalOutput")
            qt_d = nc.dram_tensor("qt_dump", [32, 2 * NQ], F32, kind="ExternalOutput")
            vx_d = nc.dram_tensor("vx_dump", [128, nkt * 65], F32, kind="ExternalOutput")
            dpool = stk.enter_context(tc.tile_pool(name="dbg", bufs=1))
            ktf = dpool.tile([32, nkt * 256], F32, tag="ktf")
            nc.vector.tensor_copy(ktf[:, :], kt_sb[:, :])
            nc.sync.dma_start(kt_d.ap(), ktf[:, :])
            qtf = dpool.tile([32, 2 * NQ], F32, tag="qtf")
            nc.vector.tensor_copy(qtf[:, :], qt_sb[:, :])
            nc.sync.dma_start(qt_d.ap(), qtf[:, :])
            vxf = dpool.tile([128, nkt * 65], F32, tag="vxf")
            nc.vector.tensor_copy(vxf[:, :], vx_sb[:, :])
            nc.sync.dma_start(vx_d.ap(), vxf[:, :])
    nc.compile()
    return nc


_cache = {}


def _programs():
    if "progs" not in _cache:
        _cache["progs"] = (build_half(CHUNKS_A), build_half(CHUNKS_B))
    return _cache["progs"]


def _host_inputs(x, W_query, W_keys, W_value):
    wqkv = np.concatenate([W_query, W_value, W_keys], axis=1).astype(np.float32)
    bmask = -240.0 * np.tril(np.ones((128, 128), np.float32), k=-1)
    wm = np.concatenate(
        [
            wqkv.reshape(NDC, 128, 192).transpose(1, 0, 2).reshape(128, NDC * 192),
            bmask,
            np.eye(128, dtype=np.float32),
        ],
        axis=1,
    ).astype(ml_dtypes.bfloat16)
    xT = np.ascontiguousarray(np.transpose(x, (0, 2, 1))).astype(ml_dtypes.bfloat16)
    NK_A = 128 * 4 * (max(CHUNKS_A) + 1)
    NK_B = 128 * 4 * (max(CHUNKS_B) + 1)
    in_A = [
        {"xT": np.ascontiguousarray(xT[b, :, :NK_A]), "wm": wm} for b in range(B)
    ]
    in_B = [
        {"xT": np.ascontiguousarray(xT[b, :, :NK_B]), "wm": wm} for b in range(B)
    ]
    return in_A, in_B


def kernel(x, W_query, W_keys, W_value, _trace=False, _tracedir=None):
    nc_a, nc_b = _programs()
    in_A, in_B = _host_inputs(x, W_query, W_keys, W_value)
    kw = {}
    if _trace:
        kw = dict(trace=True, trace_cores=[0], tmpdir=_tracedir)
    res_a = run_bass_kernel_spmd(nc_a, in_A, core_ids=[0, 1, 2, 3], **kw)
    res_b = run_bass_kernel_spmd(nc_b, in_B, core_ids=[4, 5, 6, 7], **kw)
    out = np.empty((B, N, D_OUT), np.float32)
    for b in range(B):
        for res, chunks in ((res_a, CHUNKS_A), (res_b, CHUNKS_B)):
            for ci, c in enumerate(chunks):
                out[b, 512 * c : 512 * (c + 1)] = res.results[b]["o"][
                    512 * ci : 512 * (ci + 1), :
                ]
    _cache["last_exec_ns"] = (res_a.exec_time_ns, res_b.exec_time_ns)
    return out


# revision 96
# speedup vs baseline: 1.0017x; 1.0017x over previous
"""Causal attention kernel for Trainium2, 8 NeuronCores.

Problem: x[4,4096,768] f32; Wq/Wk/Wv [768,64] f32.
  q,k,v = x@W*; S = q@k.T (causal); out = softmax(S/8)@v  -> [4,4096,64] f32.

Sharding: data-parallel over batch (4) x interleaved q-chunk split (2).
  The 8 query chunks of 512 rows are split A={0,3,4,7}, B={1,2,5,6};
  both halves get exactly half the causal score area and identical
  exp-instruction counts, so the two programs balance.
  Cores 0-3 run program A (batches 0-3), cores 4-7 run program B.

Device algorithm (per core), v3 (token-major PV, 2-slot proj ring):
  - load xT (host-transposed) [768, NK] bf16 in column waves; slot
    order puts the first kv quarter-wave, the wm constants and the
    first chunk's q wave ahead of everything else so the exp pipeline
    starts as early as the DMA stream allows.
  - projections on PE in bf16 (kv: M=128 [Wv|Wk], q: M=64), 6
    contraction passes per group, into a TWO-slot PSUM ring so the
    next group's projection overlaps this group's consumers.  Each kv
    group: ONE combined [128,g] PSUM->SBUF staging copy on DVE
    (engine cost is free-size only, so combining is free; it also
    releases the ring slot), then both fp8e4 DoubleRow k-casts on
    gpsimd off the DVE queue.  Both q casts are DVE-direct from PSUM:
    each chunk's q gates a stretch of the exp stream and the DVE
    queue has slack when its wave lands.
  - v is transposed to token-major ON PE (identity matmul into a bf16
    PSUM tile in the proj ring, Ldweights is free) instead of a DMA
    transpose: the DMA engine mutex is owned by the serial x-wave
    stream for the first ~18us, so a DMA-path transpose would stall
    PV and starve ACT.  The transpose is deferred to the next group's
    emission so a q projection can slip into the ring first.
  - scores transposed: ST[j,i] per (key tile 128 x q chunk 512) via ONE
    fp8 DoubleRow matmul per key tile (0.5 cyc/row).  fp8 rounding of
    q/k costs ~1.5% rms on the output - the only sub-bf16 step.
  - P = exp(S/8) on ACT to bf16 (no max subtraction: |S/8| <= ~7);
    the causal mask is FUSED into the score matmul: one extra
    identity-stationary matmul per diagonal tile accumulates
    -240*tril(ones,-1) onto the diagonal score block, so exp() itself
    zeroes the above-diagonal region (nothing on the exp->PV path).
  - PV is TOKEN-MAJOR: per (key tile t, token block b) a matmul with
    the P subtile [128 keys, 128 tok] STATIONARY and vx [128 keys, 65]
    moving -> o[128 tok, 65] accumulated in PSUM over t.  Matmul cost
    is output-free-size only, so this halves PV PE time, and the
    softmax denominator lands in PSUM col 64 per block (ones column
    of vx).  Only the chunk's first matmul sets start: psum zeroing
    is per 2KB zero region, later blocks land on pending-zero.
  - normalize: r = 1/denom via reciprocal_approx_fast on the strided
    PSUM denominator columns, then one broadcast tensor_tensor mult
    (DVE) straight from PSUM, then token-major DMA out [512, 64] f32;
    finishes run immediately at chunk completion, and the last
    chunk's finish is split in half so the tail is short.
  - schedule: one flat step list.  Phase 1 runs the small chunk
    entirely first (its first step SPLIT into two 256-token q halves
    so exp starts before the full q wave lands), then the big chunk
    with its pairs ordered by their kv wave's ARRIVAL (accumulation
    order is free after the start matmul), so pairs with resident
    keys run while later waves stream in.  Phase 2 merges its two
    chunks in ascending tile order with the big chunk lagged 2 pairs.
    kv groups are emitted in wave-arrival order with a 2-step
    required lookahead plus a 6-step arrived-early window; q
    projections are emitted at precomputed step indices matching
    their wave arrival.  PSUM: proj ring 2 + S 2x2 + o 2 = 8 banks.
  - a dependency-gated chain of tiny warmup matmuls keeps the cost
    model's PE p-state clock running so the real DMA-gated matmuls
    price at ramped speed.
"""

import math

import numpy as np
import ml_dtypes

import concourse.bass as bass
import concourse.bacc as bacc
import concourse.mybir as mybir
import concourse.tile as tile
from concourse.bass_utils import run_bass_kernel_spmd
from concourse.tile_rust import add_dep_helper

B, N, D_IN, D_OUT = 4, 4096, 768, 64
CHUNKS_A = [0, 3, 4, 7]
CHUNKS_B = [1, 2, 5, 6]
NDC = D_IN // 128  # 6 contraction chunks
F8 = mybir.dt.float8e4
BF16 = mybir.dt.bfloat16
F32 = mybir.dt.float32
DR = mybir.MatmulPerfMode.DoubleRow
SCALE = 0.125  # 1/sqrt(64)
WM_W = NDC * 192 + 128 + 128  # [wqkv | Bmask | I128]
STEP_NS = 1040.0  # ACT exp per full step


def _wave_order(chunks, NK):
    """512-col x-wave order: per chunk (processing order): its q-wave
    first (if not yet loaded), then any kv waves it needs, ascending."""
    waves = []
    loaded = set()

    def add(w):
        if w not in loaded and 512 * w < NK:
            waves.append(w)
            loaded.add(w)

    for c in chunks:
        add(c)  # q-wave = columns [512c, 512c+512)
        for w in range(c + 1):
            add(w)
    return waves


def build_half(chunks, debug_dump=False):
    NQ = 512 * len(chunks)
    T_need = 4 * (max(chunks) + 1)
    NK = 128 * T_need
    nkt = T_need
    nc = bacc.Bacc("TRN2", target_bir_lowering=False, debug=False)

    xT_d = nc.dram_tensor("xT", [D_IN, NK], BF16, kind="ExternalInput")
    # [wqkv 6*192 | Bmask 128 | I128 128] bf16; wqkv dc-slices [Wq|Wv|Wk]
    wm_d = nc.dram_tensor("wm", [128, WM_W], BF16, kind="ExternalInput")
    o_d = nc.dram_tensor("o", [NQ, D_OUT], F32, kind="ExternalOutput")

    from contextlib import ExitStack

    with tile.TileContext(nc) as tc, ExitStack() as stk:
        cpool = stk.enter_context(tc.tile_pool(name="const", bufs=1))
        xpool = stk.enter_context(tc.tile_pool(name="xt", bufs=1))
        jpool = stk.enter_context(tc.tile_pool(name="proj", bufs=1))
        ppool = stk.enter_context(tc.tile_pool(name="pp", bufs=3))
        fpool = stk.enter_context(tc.tile_pool(name="fin", bufs=3))

        # ---- constants / inputs ----
        w_sb = cpool.tile([128, WM_W], BF16, tag="wm")
        w3 = w_sb[:, 0 : NDC * 192].rearrange("p (c j) -> p c j", j=192)
        # Bmask[p, j] = -240 for p > j: accumulated onto diagonal score
        # blocks via one identity-stationary matmul, so exp() itself
        # zeroes the above-diagonal region (no DVE mask on the
        # exp->PV critical path)
        bm_sb = w_sb[:, NDC * 192 : NDC * 192 + 128]
        i128_sb = w_sb[:, NDC * 192 + 128 :]
        i64_sb = w_sb[0:64, NDC * 192 + 128 : NDC * 192 + 128 + 64]

        zbias = cpool.tile([128, 1], F32, tag="zbias")
        nc.vector.memset(zbias[:, :], 0.0)
        warm_sb = cpool.tile([1, 64], BF16, tag="warm")
        nc.vector.memset(warm_sb[:, :], 0.0)

        xt_sb = xpool.tile([128, NDC * NK], BF16, tag="xt")
        xt3 = xt_sb.rearrange("p (c n) -> p c n", n=NK)
        xT3d = xT_d.ap().rearrange("(c p) n -> p c n", p=128)

        # ---- DMA slot order: [w0a, wm, (phase-1 q waves if not wave 0),
        # w0b, remaining wave order].  Quarter-split of wave 0 lets the
        # first kv group start right after wm. ----
        order = _wave_order(chunks, NK)
        slots = [("x", (0, 256)), ("wm", None)]
        placed = {0}
        if chunks[0] != 0:
            # first chunk's q wave, quarter-split so its q projection can
            # start after the first half lands
            qw = chunks[0]
            slots += [("x", (512 * qw, 256)), ("x", (512 * qw + 256, 256))]
            placed.add(qw)
        slots.append(("x", (256, 256)))
        for w in order:
            if w not in placed:
                slots.append(("x", (512 * w, 512)))
                placed.add(w)
        col_eta = {}
        t_acc = 2300.0
        for kind, payload in slots:
            if kind == "wm":
                t_acc += 940.0
                continue
            g0, g = payload
            t_acc += 2185.0 * g / 512.0
            for c0 in range(g0, g0 + g, 256):
                col_eta[c0] = t_acc + 950.0  # + DMA sem propagation
        for kind, payload in slots:
            if kind == "wm":
                nc.sync.dma_start(w_sb[:, :], wm_d.ap())
            else:
                g0, g = payload
                nc.sync.dma_start(
                    xt3[:, :, g0 : g0 + g], xT3d[:, :, g0 : g0 + g]
                )

        def wave_eta_ms(col):
            # estimated arrival of the wave containing `col` (scheduler
            # hint only: stops the scheduler's DMA-blind model from
            # hoisting projections ahead of attention work)
            return col_eta[(col // 256) * 256] / 1e6

        # per-chunk q emission step index (global step counter) from the
        # arrival model: emit the q projection slightly before its wave
        # lands so the PE picks it up without head-of-line blocking
        q_arrival = {c: col_eta[512 * c + 256] for c in chunks}
        est_first_exp = q_arrival[chunks[0]] + 1700.0
        q_emit_step = {
            c: max(
                0,
                math.ceil((q_arrival[c] - 700.0 - est_first_exp) / STEP_NS),
            )
            for c in chunks
        }
        q_order = sorted(chunks, key=lambda c: q_arrival[c])

        # kv groups in WAVE-ARRIVAL order with arrival-model step
        # indices: a group whose wave has landed is emitted even if it
        # is ahead of the column-order need, so its proj/copy/cast chain
        # never queues behind a group still waiting on the DMA stream
        kv_groups = []
        g0 = 0
        while g0 < NK:
            g = min(256 if g0 < 512 else 512, NK - g0)
            arr = col_eta[g0 + g - 256]
            kv_groups.append(
                [
                    arr,
                    max(0, math.ceil((arr - est_first_exp) / STEP_NS)),
                    g0,
                    g,
                    False,
                ]
            )
            g0 += g
        kv_groups.sort(key=lambda e: (e[0], e[2]))

        # ---- projection targets ----
        kv_sb = jpool.tile([128, NK], BF16, tag="kv")  # [vT; kT] e-major
        vx_sb = jpool.tile([128, nkt * 65], BF16, tag="vx")
        vx3 = vx_sb.rearrange("p (t e) -> p t e", e=65)
        nc.gpsimd.memset(vx3[:, :, 64:65], 1.0)  # denominator ones column
        kt_sb = jpool.tile([32, nkt * 256], F8, tag="kt")
        kt4 = kt_sb.rearrange("p (t j m) -> p t j m", j=2, m=128)
        qt_sb = jpool.tile([32, 2 * NQ], F8, tag="qt")
        qt3 = qt_sb.rearrange("p (j n) -> p j n", j=2)

        pref = {}
        done = {"kv": 0, "q": set()}

        def flush_vt():
            # deferred v->token-major transpose of the last kv group (PE
            # identity transpose into the ring, bf16 PSUM out): deferring
            # keeps the ring slot free for a q projection right after the
            # group's staging copy
            pend = done.pop("vt", None)
            if pend is None:
                return
            g0, g = pend
            t0, t1 = g0 // 128, (g0 + g) // 128
            nt = t1 - t0
            vt = pref["proj"].tile([128, nt * 64], BF16, tag="proj", name="vt")
            for j in range(nt):
                nc.tensor.matmul(
                    vt[:, 64 * j : 64 * j + 64],
                    lhsT=kv_sb[0:64, g0 + 128 * j : g0 + 128 * (j + 1)],
                    rhs=i64_sb,
                    is_transpose=True,
                    start=True,
                    stop=True,
                    skip_group_check=True,
                )
            nc.vector.tensor_copy(
                vx3[:, t0:t1, 0:64],
                vt.rearrange("p (t e) -> p t e", e=64),
            )

        def emit_kv_group(g0, g, dep=None):
          flush_vt()
          with tc.tile_wait_until(wave_eta_ms(g0)):
            t0, t1 = g0 // 128, (g0 + g) // 128
            ps = pref["proj"].tile([128, 512], F32, tag="proj", name="pkv")
            for dc in range(NDC):
                mm = nc.tensor.matmul(
                    ps[:, 0:g],
                    lhsT=w3[:, dc, 64:192],
                    rhs=xt3[:, dc, g0 : g0 + g],
                    start=(dc == 0),
                    stop=(dc == NDC - 1),
                )
                if dep is not None and dc == 0:
                    # throttle: stop the scheduler (whose DMA-blind model
                    # thinks projections are ready early) from hoisting
                    # this group ahead of older attention work
                    add_dep_helper(mm.ins, dep.ins, reason="proj throttle")
            # ONE staging copy to SBUF on DVE (cost is free-size only, so
            # a combined [128,g] copy prices the same as half of it and
            # releases the proj ring slot); fp8 k casts on gpsimd after
            nc.vector.tensor_copy(kv_sb[:, g0 : g0 + g], ps[:, 0:g])
            nc.gpsimd.tensor_copy(kt4[:, t0:t1, 0, :], kv_sb[64:96, g0 : g0 + g])
            nc.gpsimd.tensor_copy(kt4[:, t0:t1, 1, :], kv_sb[96:128, g0 : g0 + g])
            done["vt"] = (g0, g)

        def emit_kv_cols(c_lo, c_hi, dep=None, arrived_only=False):
            # emit unemitted groups overlapping [c_lo, c_hi), scanning in
            # wave-arrival order so a group with resident data never
            # queues behind one still waiting on the DMA stream
            for e in kv_groups:
                arr, _, g0, g, emitted = e
                if emitted or g0 + g <= c_lo or g0 >= c_hi:
                    continue
                if arrived_only and arr > e_now[0]:
                    continue
                emit_kv_group(g0, g, dep=dep)
                e[4] = True

        def emit_kv_upto(tok, dep=None):
            emit_kv_cols(0, min(tok, NK), dep=dep)

        def emit_q_half(ci, qc0, h, ps=None):
          """Project + cast one 256-col half of a q chunk (startup path:
          the half can start as soon as its quarter-wave lands).  `ps`
          may supply a spare PSUM region (e.g. the unused second bank of
          an S half-tile) to keep the proj ring free for kv groups."""
          with tc.tile_wait_until(wave_eta_ms(qc0 + 256 * h)):
            ql0 = 512 * ci + 256 * h
            if ps is None:
                ps = pref["proj"].tile([64, 256], F32, tag="proj", name="pqh")
            for dc in range(NDC):
                nc.tensor.matmul(
                    ps[:, :],
                    lhsT=w3[:, dc, 0:64],
                    rhs=xt3[:, dc, qc0 + 256 * h : qc0 + 256 * h + 256],
                    start=(dc == 0),
                    stop=(dc == NDC - 1),
                )
            nc.vector.tensor_copy(qt3[:, 0, ql0 : ql0 + 256], ps[0:32, :])
            nc.vector.tensor_copy(qt3[:, 1, ql0 : ql0 + 256], ps[32:64, :])

        def emit_q(ci, qc0):
          # steady-state q: one full-width projection; staging copy on
          # DVE (releases the ring slot fast), fp8 casts on gpsimd
          if ci in done["q"]:
              return
          done["q"].add(ci)
          with tc.tile_wait_until(wave_eta_ms(qc0 + 256)):
            ql0 = 512 * ci
            ps = pref["proj"].tile([64, 512], F32, tag="proj", name="pq")
            for dc in range(NDC):
                nc.tensor.matmul(
                    ps[:, :],
                    lhsT=w3[:, dc, 0:64],
                    rhs=xt3[:, dc, qc0 : qc0 + 512],
                    start=(dc == 0),
                    stop=(dc == NDC - 1),
                )
            # both fp8 casts DVE-direct from PSUM: each chunk's q gates a
            # stretch of the exp stream, and the DVE queue has slack when
            # its wave lands — cutting the bf16-staging + gpsimd hop
            # shortens the wave->S chain by ~1us
            nc.vector.tensor_copy(qt3[:, 0, ql0 : ql0 + 512], ps[0:32, :])
            nc.vector.tensor_copy(qt3[:, 1, ql0 : ql0 + 512], ps[32:64, :])

        def q_check(gsi):
            for c in q_order:
                ci = chunks.index(c)
                if ci not in done["q"] and q_emit_step[c] <= gsi:
                    emit_q(ci, 512 * c)

        # ---- psum pools: proj ring 2 + s 2x2 + o 2 = 8 banks ----
        pref["proj"] = stk.enter_context(
            tc.tile_pool(name="ppsum", bufs=2, space="PSUM")
        )
        # p-state warmup chain (see module docstring)
        wsrc = warm_sb
        for wi in range(8):
            warm_ps = pref["proj"].tile([1, 64], F32, tag="proj", name=f"w{wi}")
            nc.tensor.matmul(
                warm_ps[:, :], lhsT=wsrc[:, 0:1], rhs=wsrc[:, :],
                start=True, stop=True,
            )
            wsrc = cpool.tile([1, 64], BF16, tag=f"warm{wi}")
            nc.vector.tensor_copy(wsrc[:, :], warm_ps[:, :])
        spsum = stk.enter_context(tc.tile_pool(name="spsum", bufs=2, space="PSUM"))
        opsum = stk.enter_context(tc.tile_pool(name="opsum", bufs=2, space="PSUM"))

        # ---- attention: two phases, chunks merged by tile order ----
        class Ck:
            def __init__(self, ci, c):
                self.ci, self.c = ci, c
                self.qc0 = 512 * c
                self.ql0 = 512 * ci
                self.T_c = 4 * (c + 1)
                self.npair = self.T_c // 2
                self.remaining = self.npair
                self.o_tile = None
                self.o3 = None
                self.s_cur = None

        def emit_s(ck, pi):
            emit_q(ck.ci, ck.qc0)  # idempotent: q must precede its S
            t0 = 2 * pi
            i0g = max(0, 128 * t0 - ck.qc0)
            s_tile = spsum.tile([128, 1024], F32, tag="s")
            for tl in range(2):
                nc.tensor.matmul(
                    s_tile[:, 512 * tl + i0g : 512 * tl + 512],
                    lhsT=kt4[:, t0 + tl, :, :],
                    rhs=qt3[:, :, ck.ql0 + i0g : ck.ql0 + 512],
                    start=True,
                    stop=True,
                    perf_mode=DR,
                )
                dcol = 128 * (t0 + tl) - ck.qc0
                if 0 <= dcol < 512:  # fused causal mask on diag block
                    nc.tensor.matmul(
                        s_tile[:, 512 * tl + dcol : 512 * tl + dcol + 128],
                        lhsT=i128_sb,
                        rhs=bm_sb,
                        start=False,
                        stop=True,
                        skip_group_check=True,
                    )
            return s_tile

        def emit_s_half(ck, s_half, h):
            # first program step only: scores for one 256-token q half
            # (own [128,512] tile per half so exp h0 has no false dep on
            # the h1 score matmuls)
            for tl in range(2):
                nc.tensor.matmul(
                    s_half[:, 256 * tl : 256 * tl + 256],
                    lhsT=kt4[:, tl, :, :],
                    rhs=qt3[:, :, ck.ql0 + 256 * h : ck.ql0 + 256 * h + 256],
                    start=True,
                    stop=True,
                    perf_mode=DR,
                )
                dcol = 128 * tl - ck.qc0
                if 0 <= dcol < 512 and 256 * h <= dcol < 256 * h + 256:
                    nc.tensor.matmul(
                        s_half[
                            :,
                            256 * tl + dcol - 256 * h : 256 * tl + dcol - 256 * h + 128,
                        ],
                        lhsT=i128_sb,
                        rhs=bm_sb,
                        start=False,
                        stop=True,
                        skip_group_check=True,
                    )

        def emit_fin(ck, blo, bhi):
            """Normalize + store token blocks [blo, bhi) of chunk ck."""
            nb = bhi - blo
            o3 = ck.o3
            r_tile = fpool.tile([128, 4], F32, tag="r", name="r")
            nc.vector.reciprocal_approx_fast(
                r_tile[:, 0:nb], o3[:, blo:bhi, 64]
            )
            n_tile = fpool.tile([128, 256], F32, tag="n", name="n")
            n3 = n_tile.rearrange("p (b e) -> p b e", e=64)
            nc.vector.tensor_tensor(
                n3[:, blo:bhi, :],
                o3[:, blo:bhi, 0:64],
                r_tile[:, 0:nb, None].broadcast_to([128, nb, 64]),
                op=mybir.AluOpType.mult,
            )
            out_ap = o_d.ap()[
                ck.ql0 + 128 * blo : ck.ql0 + 128 * bhi, :
            ].rearrange("(b p) e -> p b e", p=128)
            nc.sync.dma_start(
                out_ap,
                n_tile.rearrange("p (b e) -> p b e", e=64)[:, blo:bhi, :],
            )

        def process(ck, pi, s_next_step):
            """Emit exp/mask/PV for (ck, pi); S for s_next_step emitted
            first so the PE runs ahead of ACT."""
            s_cur = ck.s_cur
            if s_next_step is not None:
                nck, npi = s_next_step
                nck.s_cur = emit_s(nck, npi)
            t0, t1 = 2 * pi, 2 * pi + 1
            i0g = max(0, 128 * t0 - ck.qc0)
            p_tile = ppool.tile([128, 1024], BF16, tag="p")
            p3 = p_tile.rearrange("p (t i) -> p t i", i=512)
            if i0g == 0:
                s_ap, p_ap = s_cur[:, :], p_tile[:, :]
            else:
                s_ap = s_cur.rearrange("p (t i) -> p t i", i=512)[:, :, i0g:512]
                p_ap = p3[:, :, i0g:512]
            exp_inst = nc.scalar.activation(
                p_ap, s_ap, mybir.ActivationFunctionType.Exp,
                bias=zbias[:, :], scale=SCALE,
            )
            for tl, t in ((0, t0), (1, t1)):
                dcol = 128 * t - ck.qc0
                # token-major PV: P subtile stationary, vx moving.
                # start only on the tile's FIRST matmul: start marks the
                # whole 2KB psum zero region, so later blocks' first
                # writes land on pending-zero (one start per bank).
                b_min = max(0, dcol // 128)
                for b in range(b_min, 4):
                    nc.tensor.matmul(
                        ck.o_tile[:, 65 * b : 65 * b + 65],
                        lhsT=p3[:, tl, 128 * b : 128 * b + 128],
                        rhs=vx3[:, t, :],
                        start=(t == 0 and b == 0),
                        stop=(t == 4 * ck.c + b),
                        skip_group_check=True,
                    )
            return exp_inst

        def process_first_split(ck, s_next_step):
            """First program step, pipelined in two 256-token q halves:
            exp of half 0 runs while half 1's q/S chain completes."""
            s_halves = ck.s_cur
            p_tile = ppool.tile([128, 1024], BF16, tag="p")
            p3 = p_tile.rearrange("p (t i) -> p t i", i=512)
            exps = []
            for h in (0, 1):
                if h == 1 and s_next_step is not None:
                    nck, npi = s_next_step
                    nck.s_cur = emit_s(nck, npi)
                exps.append(
                    nc.scalar.activation(
                        p3[:, :, 256 * h : 256 * h + 256],
                        s_halves[h].rearrange("p (t i) -> p t i", i=256),
                        mybir.ActivationFunctionType.Exp,
                        bias=zbias[:, :],
                        scale=SCALE,
                    )
                )
                for tl, t in ((0, 0), (1, 1)):
                    dcol = 128 * t - ck.qc0
                    b_min = max(0, dcol // 128)
                    for b in (2 * h, 2 * h + 1):
                        if b < b_min:
                            continue
                        nc.tensor.matmul(
                            ck.o_tile[:, 65 * b : 65 * b + 65],
                            lhsT=p3[:, tl, 128 * b : 128 * b + 128],
                            rhs=vx3[:, t, :],
                            start=(t == 0 and b == 0),
                            stop=(t == 4 * ck.c + b),
                            skip_group_check=True,
                        )
            return exps

        cks = [Ck(ci, c) for ci, c in enumerate(chunks)]
        # phase 1 sequential: small chunk entirely first — its steps run
        # while the big chunk's q/kv waves are still arriving.  phase 2
        # merged ascending tile order, big chunk lagged 2 pairs (its q
        # wave arrives later); small first on ties.  One flat list so
        # the S pipeline chains across the phase boundary.
        big2 = max(cks[2:4], key=lambda k: k.npair)
        # phase-1 big chunk pairs ordered by their kv wave's ARRIVAL
        # (accumulation order is free once the start matmul ran): pairs
        # whose keys are already resident run while later waves stream in
        p1big = sorted(
            range(cks[1].npair),
            key=lambda p: (col_eta.get(256 * p, 0.0), p),
        )
        steps = (
            [(cks[0], pi) for pi in range(cks[0].npair)]
            + [(cks[1], pi) for pi in p1big]
            + sorted(
                [(ck, pi) for ck in cks[2:4] for pi in range(ck.npair)],
                key=lambda s: (s[1] + (2 if s[0] is big2 else 0), s[0].c),
            )
        )
        for ck in cks:
            ck.o_tile = opsum.tile([128, 260], F32, tag="ot", name=f"o{ck.ci}")
            ck.o3 = ck.o_tile.rearrange("p (b e) -> p b e", e=65)

        # prime the first (split) step
        ck0 = steps[0][0]
        emit_kv_upto(128 * (2 * steps[0][1] + 2))
        done["q"].add(ck0.ci)
        emit_q_half(ck0.ci, ck0.qc0, 0)
        s_h0 = spsum.tile([128, 512], F32, tag="s", name="sh0")
        emit_s_half(ck0, s_h0, 0)
        # second kv group next: with h1's projection routed into the
        # spare second bank of the s_h1 tile, the proj ring is free and
        # kv0b's chain overlaps the whole h1 q chain
        emit_kv_upto(512)
        s_h1 = spsum.tile([128, 1024], F32, tag="s", name="sh1")
        emit_q_half(ck0.ci, ck0.qc0, 1, ps=s_h1[0:64, 512:768])
        emit_s_half(ck0, s_h1[:, 0:512], 1)
        ck0.s_cur = (s_h0, s_h1[:, 0:512])

        exp_hist = []
        e_now = [0.0]
        for si, (ck, pi) in enumerate(steps):
            nxt = steps[si + 1] if si + 1 < len(steps) else None
            if nxt is not None:
                # two-step kv lookahead (a group's proj/copy/cast chain
                # is ~2.5us), plus arrived-early groups needed within
                # the next six steps
                dep = exp_hist[-9] if len(exp_hist) >= 9 else None
                e_now[0] = est_first_exp + STEP_NS * si
                for k in (1, 2):
                    st = steps[min(si + k, len(steps) - 1)]
                    emit_kv_cols(256 * st[1], 256 * st[1] + 256, dep=dep)
                for k in range(3, 7):
                    st = steps[min(si + k, len(steps) - 1)]
                    emit_kv_cols(
                        256 * st[1],
                        256 * st[1] + 256,
                        dep=dep,
                        arrived_only=True,
                    )
                q_check(si)
            flush_vt()
            if si == 0:
                exp_hist.extend(process_first_split(ck, nxt))
            else:
                exp_hist.append(process(ck, pi, nxt))
            ck.remaining -= 1
            if ck is cks[-1] and ck.remaining == 1:
                # early half-finish of the final chunk: token blocks 0-1
                # are complete after its second-to-last processed pair
                # (processing order is ascending there)
                emit_fin(ck, 0, 2)
                ck.fin_half = True
            if ck.remaining == 0:
                if getattr(ck, "fin_half", False):
                    emit_fin(ck, 2, 4)
                else:
                    emit_fin(ck, 0, 4)

        if debug_dump:
            kt_d = nc.dram_tensor("kt_dump", [32, nkt * 256], F32, kind="ExternalOutput")
            qt_d = nc.dram_tensor("qt_dump", [32, 2 * NQ], F32, kind="ExternalOutput")
            vx_d = nc.dram_tensor("vx_dump", [128, nkt * 65], F32, kind="ExternalOutput")
            dpool = stk.enter_context(tc.tile_pool(name="dbg", bufs=1))
            ktf = dpool.tile([32, nkt * 256], F32, tag="ktf")
            nc.vector.tensor_copy(ktf[:, :], kt_sb[:, :])
            nc.sync.dma_start(kt_d.ap(), ktf[:, :])
            qtf = dpool.tile([32, 2 * NQ], F32, tag="qtf")
            nc.vector.tensor_copy(qtf[:, :], qt_sb[:, :])
            nc.sync.dma_start(qt_d.ap(), qtf[:, :])
            vxf = dpool.tile([128, nkt * 65], F32, tag="vxf")
            nc.vector.tensor_copy(vxf[:, :], vx_sb[:, :])
            nc.sync.dma_start(vx_d.ap(), vxf[:, :])
    nc.compile()
    return nc


_cache = {}


def _programs():
    if "progs" not in _cache:
        _cache["progs"] = (build_half(CHUNKS_A), build_half(CHUNKS_B))
    return _cache["progs"]


def _host_inputs(x, W_query, W_keys, W_value):
    wqkv = np.concatenate([W_query, W_value, W_keys], axis=1).astype(np.float32)
    bmask = -240.0 * np.tril(np.ones((128, 128), np.float32), k=-1)
    wm = np.concatenate(
        [
            wqkv.reshape(NDC, 128, 192).transpose(1, 0, 2).reshape(128, NDC * 192),
            bmask,
            np.eye(128, dtype=np.float32),
        ],
        axis=1,
    ).astype(ml_dtypes.bfloat16)
    xT = np.ascontiguousarray(np.transpose(x, (0, 2, 1))).astype(ml_dtypes.bfloat16)
    NK_A = 128 * 4 * (max(CHUNKS_A) + 1)
    NK_B = 128 * 4 * (max(CHUNKS_B) + 1)
    in_A = [
        {"xT": np.ascontiguousarray(xT[b, :, :NK_A]), "wm": wm} for b in range(B)
    ]
    in_B = [
        {"xT": np.ascontiguousarray(xT[b, :, :NK_B]), "wm": wm} for b in range(B)
    ]
    return in_A, in_B


def kernel(x, W_query, W_keys, W_value, _trace=False, _tracedir=None):
    nc_a, nc_b = _programs()
    in_A, in_B = _host_inputs(x, W_query, W_keys, W_value)
    kw = {}
    if _trace:
        kw = dict(trace=True, trace_cores=[0], tmpdir=_tracedir)
    res_a = run_bass_kernel_spmd(nc_a, in_A, core_ids=[0, 1, 2, 3], **kw)
    res_b = run_bass_kernel_spmd(nc_b, in_B, core_ids=[4, 5, 6, 7], **kw)
    out = np.empty((B, N, D_OUT), np.float32)
    for b in range(B):
        for res, chunks in ((res_a, CHUNKS_A), (res_b, CHUNKS_B)):
            for ci, c in enumerate(chunks):
                out[b, 512 * c : 512 * (c + 1)] = res.results[b]["o"][
                    512 * ci : 512 * (ci + 1), :
                ]
    _cache["last_exec_ns"] = (res_a.exec_time_ns, res_b.exec_time_ns)
    return out
